# revision 9
# baseline (speedup 1.0000x reference)
"""CostVolumeLayer Trainium2 kernel, v7.

v4 -> v6: 8-row tgt chunks (first strip's windows ready a few us sooner;
4.7 KB/partition descriptors keep packets efficient); tgt host-padded in
x only with the S top/bottom rows memset on-device (row-contiguous chunk
DMAs preserved, 0.6 MB fewer reads); last strip's output quarters
alternate gpsimd/sync so the final drain uses both queues. The v5
per-subgroup write trimming (288 B packets) collapsed DMA throughput
and was abandoned.

Below is the v4 description.
"""
"""CostVolumeLayer Trainium2 kernel, v4.

v2 -> v4 changes (driven by the v2/v3 traces, 132us/138us):
  - Evacuate PSUM in 2-tile pairs ([128, 384] per bank) so the per-op
    overhead of DVE/ACT evacuation is paid half as often (evac lagged the
    matmul stream per strip in v2).
  - src strip 0 is loaded in quarters so the first matmuls start ~6us
    earlier (v2 spent 20us before the first matmul waiting on the full
    first strip behind the tgt-chunk round-robin).
  - The last strip's output DMA is split into quarters to shorten the
    drain tail.
  - Output DMAs alternate between the SWDGE (gpsimd) and SP HWDGE (sync)
    queues by strip so the write-heavy drain phase uses two queues.
  - tgt stays HOST-PADDED: v3 loaded it unpadded with on-device border
    memsets and the interior DMA shattered into 573 B packets (24 row
    segments per partition per chunk), slowing the read path far more
    than the 0.93 MB saved.

Core algorithm (from v2): 16x8 position tiles, 4 col-grouped matmuls per
tile (tile_position=(0,32j)) computing only the group's 12x16=192-column
band of the tgt window via 2-D rhs APs into the SBUF-resident padded tgt;
banded Gram written bf16 (write amplification 2.37x); host desheres.
"""

import sys

for _p in ("/opt/trn_rl_repo",):
    if _p not in sys.path:
        sys.path.insert(0, _p)

import numpy as np
import ml_dtypes

import concourse.mybir as mybir
import concourse.tile as tile
from concourse import bacc
from concourse.bass_utils import run_bass_kernel_spmd

B, C, S = 8, 128, 4
H, W = 160, 288
TY, TX = 16, 8                       # tile = 16x8 = 128 positions
GY = 4                               # y-rows per col-group (32 partitions)
NG = TY // GY                        # 4 col groups
WIN_X = TX + 2 * S                   # 16 window cols
BAND_Y = GY + 2 * S                  # 12 window rows per group band
BAND = BAND_Y * WIN_X                # 192 PSUM cols per tile
TGT_CHUNK = 8                        # tgt rows per chunk DMA
N_CORES = 8

BF16 = mybir.dt.bfloat16
NP_BF16 = ml_dtypes.bfloat16


def _displacements(s):
    d = [(0, 0)]
    for i in range(1, s + 1):
        d += [(-i, 0), (i, 0), (0, -i), (0, i)]
        for j in range(1, s + 1):
            d += [(-i, -j), (i, j), (-i, j), (i, -j)]
    return d


DISPLACEMENTS = _displacements(S)


def _build_bass(h=H, w=W, n_devices=N_CORES):
    nstrip = h // TY
    nxt = w // TX
    hp, wp = h + 2 * S, w + 2 * S
    n_chunks = (hp + TGT_CHUNK - 1) // TGT_CHUNK
    assert nxt % 2 == 0
    nsplit0 = 4 if nxt % 4 == 0 else 2   # src strip-0 / last-out split

    nc = bacc.Bacc(
        "TRN2",
        target_bir_lowering=False,
        debug=False,
        num_devices=n_devices,
    )
    # src pre-tiled on host: [C, nstrip, nxt*128] so each tile's lhsT is
    # one contiguous 128-element slice (pos = ylocal*8 + xlocal).
    src_t = nc.dram_tensor(
        "src", [C, nstrip, nxt * TY * TX], BF16, kind="ExternalInput"
    ).ap()
    tgt_t = nc.dram_tensor("tgtp", [C, h, wp], BF16, kind="ExternalInput").ap()
    out_t = nc.dram_tensor(
        "gram", [nstrip, C, nxt * BAND], BF16, kind="ExternalOutput"
    ).ap()

    with tile.TileContext(nc) as tc:
        with (
            tc.tile_pool(name="tgtres", bufs=1) as tgt_pool,
            tc.tile_pool(name="srcstrip", bufs=3) as src_pool,
            tc.tile_pool(name="outstrip", bufs=3) as out_pool,
            tc.tile_pool(name="psum", bufs=8, space="PSUM") as psum_pool,
        ):
            # One SBUF-resident padded tgt, loaded in row chunks so early
            # strips only depend on the first chunks. Matmul windows that
            # span chunk boundaries rely on Tile's range-based hazard
            # tracking for DMA->matmul deps.
            tgt_tile = tgt_pool.tile([C, hp * wp], BF16)
            tgt_view = tgt_tile.rearrange("p (y x) -> p y x", x=wp)
            # tgt is x-padded on the host; the S top/bottom padded rows are
            # memset here so chunk DMAs stay fully contiguous per partition.
            nc.gpsimd.memset(tgt_view[:, 0:S], 0.0)
            nc.gpsimd.memset(tgt_view[:, S + h : hp], 0.0)
            for ci in range(n_chunks):
                r0, r1 = ci * TGT_CHUNK, min((ci + 1) * TGT_CHUNK, hp)
                ir0, ir1 = max(r0, S), min(r1, S + h)
                if ir0 < ir1:
                    nc.scalar.dma_start(
                        tgt_view[:, ir0:ir1], tgt_t[:, ir0 - S : ir1 - S]
                    )

            # Work in half-strip segments (when the tile count allows):
            # finer pipeline granularity shortens both the ramp-in (first
            # output bytes fly sooner) and the drain tail (last segment is
            # half as much evac + write work).
            seg = nxt // 2 if (nxt // 2) % 2 == 0 else nxt
            segments = [
                (s, t0, t0 + seg)
                for s in range(nstrip)
                for t0 in range(0, nxt, seg)
            ]
            nseg = len(segments)
            for si, (s, t0, t1) in enumerate(segments):
                ntile = t1 - t0
                src_tile = src_pool.tile([C, ntile * TY * TX], BF16)
                nsplit = 2 if si == 0 and ntile % 2 == 0 else 1
                qt = ntile // nsplit * TY * TX
                for q in range(nsplit):
                    nc.sync.dma_start(
                        src_tile[:, q * qt : (q + 1) * qt],
                        src_t[
                            :, s, t0 * TY * TX + q * qt : t0 * TY * TX + (q + 1) * qt
                        ],
                    )
                src_view = src_tile.rearrange("p (t m) -> p t m", m=TY * TX)

                out_tile = out_pool.tile([C, ntile * BAND], BF16)

                for tp in range(ntile // 2):
                    # two position-tiles share one PSUM bank -> one evac
                    ps = psum_pool.tile([C, 2 * BAND], mybir.dt.float32)
                    for ti in range(2):
                        t = t0 + 2 * tp + ti
                        for j in range(NG):
                            nc.tensor.matmul(
                                ps[
                                    32 * j : 32 * (j + 1),
                                    ti * BAND : ti * BAND + BAND,
                                ],
                                lhsT=src_view[
                                    :, 2 * tp + ti, 32 * j : 32 * (j + 1)
                                ],
                                rhs=tgt_view[
                                    :,
                                    TY * s + GY * j : TY * s + GY * j + BAND_Y,
                                    TX * t : TX * t + WIN_X,
                                ],
                                start=True,
                                stop=True,
                                tile_position=(0, 32 * j),
                            )
                    # Alternate evacuation engine so DVE and ACT each take
                    # half the pairs and run concurrently.
                    dst = out_tile[:, 2 * tp * BAND : (2 * tp + 2) * BAND]
                    if tp % 2 == 0:
                        nc.vector.tensor_scalar_mul(dst, ps[:], 1.0 / C)
                    else:
                        nc.scalar.mul(dst, ps[:], 1.0 / C)

                # Writes avoid the sync ring entirely (HWDGE rings are
                # FIFO: a queued write would delay the next src segment
                # load). Early segments write via SWDGE (gpsimd); once the
                # tgt chunks have drained from the scalar ring later
                # segments write there. The last segment drains in halves
                # on both write queues.
                nout = 2 if si == nseg - 1 and ntile % 2 == 0 else 1
                part = (ntile // nout) * BAND
                for q in range(nout):
                    if si == nseg - 1:
                        out_eng = nc.gpsimd if q % 2 == 0 else nc.scalar
                    else:
                        out_eng = nc.gpsimd if si < nseg // 2 else nc.scalar
                    out_eng.dma_start(
                        out_t[
                            s, :, t0 * BAND + q * part : t0 * BAND + (q + 1) * part
                        ],
                        out_tile[:, q * part : (q + 1) * part],
                    )

    nc.compile()
    return nc


_NC = None


def _get_nc():
    global _NC
    if _NC is None:
        _NC = _build_bass()
    return _NC


def _run_device(src_bf, tgtp_bf, **run_kwargs):
    nc = _get_nc()
    in_maps = [{"src": src_bf[b], "tgtp": tgtp_bf[b]} for b in range(B)]
    return run_bass_kernel_spmd(nc, in_maps, core_ids=list(range(N_CORES)), **run_kwargs)


def _pad_tgt(tgt, h=H, w=W):
    # x-padded only; the S top/bottom rows are memset on-device.
    b, c = tgt.shape[0], tgt.shape[1]
    tgtp = np.zeros((b, c, h, w + 2 * S), NP_BF16)
    tgtp[:, :, :, S : S + w] = tgt.astype(NP_BF16)
    return tgtp


def _pretile_src(src, h=H, w=W):
    """[B, C, h, w] -> [B, C, nstrip, nxt*TY*TX] bf16, pos = ylocal*TX+xlocal."""
    b, c = src.shape[0], src.shape[1]
    nstrip, nxt = h // TY, w // TX
    return np.ascontiguousarray(
        src.astype(NP_BF16)
        .reshape(b, c, nstrip, TY, nxt, TX)
        .transpose(0, 1, 2, 4, 3, 5)
        .reshape(b, c, nstrip, nxt * TY * TX)
    )


def _deshear(gram, h=H, w=W):
    """gram: [B, nstrip, 128, nxt*BAND] (any float dtype) -> [B, 81, h, w] fp32.

    gram[b, s, p, t*BAND + wr*WIN_X + wx] with p = ylocal*TX + xlocal holds
    (1/C) * sum_c src[c, TY*s+ylocal, TX*t+xlocal]
                * tgtp[c, TY*s + GY*(ylocal//GY) + wr, TX*t + wx]
    For displacement (dy,dx): wr = ylocal%GY + dy + S, wx = xlocal + dx + S.
    """
    b = gram.shape[0]
    nstrip, nxt = h // TY, w // TX
    g = np.asarray(gram, dtype=np.float32).reshape(
        b, nstrip, TY, TX, nxt, BAND_Y, WIN_X
    )
    out = np.empty((b, len(DISPLACEMENTS), h, w), np.float32)
    yy = np.arange(TY)[:, None]
    xx = np.arange(TX)[None, :]
    for k, (dy, dx) in enumerate(DISPLACEMENTS):
        # fancy dims (yy, xx) land first: v = [TY, TX, b, nstrip, nxt]
        v = g[:, :, yy, xx, :, (yy % GY) + dy + S, xx + dx + S]
        out[:, k] = v.transpose(2, 3, 0, 4, 1).reshape(b, h, w)
    return out


def kernel(src, tgt, _profile_out=None):
    src = np.asarray(src)
    tgt = np.asarray(tgt)
    assert src.shape == (B, C, H, W) and tgt.shape == (B, C, H, W)

    src_bf = _pretile_src(src)
    tgtp_bf = _pad_tgt(tgt)

    kw = {}
    if _profile_out is not None:
        kw["trace"] = True
        if _profile_out.get("tmpdir"):
            kw["tmpdir"] = _profile_out["tmpdir"]
    res = _run_device(src_bf, tgtp_bf, **kw)
    if _profile_out is not None:
        _profile_out.update(
            exec_time_ns=res.exec_time_ns,
            mean_exec_time_ns=res.mean_exec_time_ns,
        )

    gram = np.stack([res.results[b]["gram"] for b in range(B)])
    return _deshear(gram)
